# revision 6
# baseline (speedup 1.0000x reference)
"""Trainium2 Bass kernel for the Kendall-correlation few-shot scoring layer.

Math: score[n,s] = mean_p(2*sigmoid(qd[n,p]*sd[s,p]) - 1)/T
              = mean_p tanh(qd[n,p]*sd[s,p]/2)/T
with qd/sd the channel-pair differences of spatially pooled query/support
features.  The products x = qd*sd lie in [-0.52, 0.52], so tanh(x/2) is
replaced by a fitted odd polynomial a1*y + a3*y^3 (y=x/2), which turns the
whole (Nq, Ns, P) tensor into a handful of matmuls:

  term1 = qf @ [g*M1 @ proto.T]            (M1 = S^T S = c*I - J, rank-c exact)
  term3 = (qd^3) @ (k3 * sd^3).T           (pair-space matmul, bf16)

Sharding: data-parallel over queries, 188 rows/core x 8 cores (1500 padded
to 1504); support prototypes replicated.
"""

import os

import numpy as np
import ml_dtypes

import concourse.bass as bass
import concourse.bacc as bacc
import concourse.tile as tile
from concourse import mybir
from concourse.bass_utils import run_bass_kernel_spmd
from concourse._compat import with_exitstack
from contextlib import ExitStack

F32 = mybir.dt.float32
BF16 = mybir.dt.bfloat16

NCORES = 8
QPC = 188            # queries per core (8*188 = 1504 >= 1500)
C = 64               # channels
HW = 25              # spatial h*w
CS = C * HW          # 1600
NS = 20              # t * way prototypes
NPAIR = 2016         # C*(C-1)/2
NPAD = 2048          # padded pairs = 16 chunks of 128
NCHUNK = NPAD // 128
T_CONST = 0.0125
BETA = 1.0
# fitted odd-polynomial coefficients for tanh(y) on |y| <= 0.32
A1 = 0.999958
A3 = -0.32370582
G = 1.0 / (NPAIR * T_CONST)
K3 = G * A3 / 8.0    # coefficient on qd^3 * sd^3

QBLOCKS = [(0, 128), (128, 60)]   # query sub-blocks (PE stationary M <= 128)
WCH = 4                           # QDT chunks per wave


@with_exitstack
def _body(ctx: ExitStack, tc, q, smat, protoT, pm_hi, pm_lo, ident, out):
    nc = tc.nc
    SQ = mybir.ActivationFunctionType.Square
    CP = mybir.ActivationFunctionType.Copy
    AX = mybir.AxisListType.X

    const = ctx.enter_context(tc.tile_pool(name="const", bufs=1))
    work = ctx.enter_context(tc.tile_pool(name="work", bufs=1))
    xpool = ctx.enter_context(tc.tile_pool(name="xp", bufs=2))
    ps_misc = ctx.enter_context(tc.tile_pool(name="psm", bufs=1, space="PSUM"))
    ps_qdt = ctx.enter_context(tc.tile_pool(name="psq", bufs=2, space="PSUM"))
    ps_score = ctx.enter_context(tc.tile_pool(name="pss", bufs=1, space="PSUM"))

    # ---- constant loads -------------------------------------------------
    smat_sb = const.tile([C, NPAD], BF16)
    nc.sync.dma_start(smat_sb[:], smat[:])
    protoT_sb = const.tile([C, NS], BF16)
    nc.sync.dma_start(protoT_sb[:], protoT[:])
    pm_hi_sb = const.tile([C, NS], BF16)
    nc.sync.dma_start(pm_hi_sb[:], pm_hi[:])
    pm_lo_sb = const.tile([C, NS], BF16)
    nc.sync.dma_start(pm_lo_sb[:], pm_lo[:])
    ident_sb = const.tile([128, 128], F32)
    nc.sync.dma_start(ident_sb[:], ident[:])

    # ---- query rows -----------------------------------------------------
    q0 = work.tile([128, CS], F32, tag="q0")
    nc.sync.dma_start(q0[:], q[0:128, :])
    q1 = work.tile([60, CS], F32, tag="q1")
    nc.sync.dma_start(q1[:], q[128:QPC, :])

    # ---- support side: sd^3 coefficients (tiny) -------------------------
    # SDT[p, s] = sum_c smat[c, p] * protoT[c, s], chunks stacked on free dim
    sdt_ps = ps_misc.tile([128, NCHUNK * NS], F32, tag="sdt")
    for ci in range(NCHUNK):
        nc.tensor.matmul(
            sdt_ps[:, ci * NS:(ci + 1) * NS],
            smat_sb[:, ci * 128:(ci + 1) * 128],
            protoT_sb[:],
            start=True, stop=True,
        )
    t2_sd = work.tile([128, NCHUNK * NS], BF16, tag="t2sd")
    nc.scalar.activation(t2_sd[:], sdt_ps[:], SQ)              # sd^2
    s1a = work.tile([128, NCHUNK * NS], BF16, tag="s1a")
    nc.scalar.activation(s1a[:], sdt_ps[:], CP, bias=0.0, scale=float(K3))
    s3 = work.tile([128, NCHUNK * NS], BF16, tag="s3")
    nc.vector.tensor_mul(s3[:], s1a[:], t2_sd[:])              # k3 * sd^3

    # ---- query pooling (sum over spatial, fp32) -------------------------
    qs0 = work.tile([128, C], F32, tag="qs0")
    nc.vector.reduce_sum(qs0[:], q0[:].rearrange("p (c s) -> p c s", s=HW), axis=AX)
    qs1 = work.tile([60, C], F32, tag="qs1")
    nc.vector.reduce_sum(qs1[:], q1[:].rearrange("p (c s) -> p c s", s=HW), axis=AX)

    # ---- transpose to [C, QPC], scale 1/HW, split hi/lo bf16 ------------
    qsT_ps = ps_misc.tile([C, QPC], F32, tag="qsT")
    nc.tensor.transpose(qsT_ps[:, 0:128], qs0[:], ident_sb[:])
    nc.tensor.transpose(qsT_ps[:, 128:QPC], qs1[:], ident_sb[0:60, 0:60])
    qsT = work.tile([C, QPC], F32, tag="qsTs")
    nc.scalar.activation(qsT[:], qsT_ps[:], CP, bias=0.0, scale=1.0 / HW)
    hi = work.tile([C, QPC], BF16, tag="hi")
    nc.vector.tensor_copy(hi[:], qsT[:])
    lo = work.tile([C, QPC], BF16, tag="lo")
    nc.vector.tensor_sub(lo[:], qsT[:], hi[:])

    # ---- score accumulation in PSUM ------------------------------------
    sc0 = ps_score.tile([128, NS], F32, tag="sc0")
    sc1 = ps_score.tile([60, NS], F32, tag="sc1")
    scs = [sc0, sc1]

    # term1: split-precision rank-C path
    for (qo, qn), sc in zip(QBLOCKS, scs):
        nc.tensor.matmul(sc[:], hi[:, qo:qo + qn], pm_hi_sb[:], start=True, stop=False)
        nc.tensor.matmul(sc[:], hi[:, qo:qo + qn], pm_lo_sb[:], start=False, stop=False)
        nc.tensor.matmul(sc[:], lo[:, qo:qo + qn], pm_hi_sb[:], start=False, stop=False)

    # term3: pair-space waves
    for w in range(NCHUNK // WCH):
        qdt = ps_qdt.tile([128, WCH, 256], F32, tag="qdt")
        for k in range(WCH):
            ci = w * WCH + k
            nc.tensor.matmul(
                qdt[:, k, 0:QPC],
                smat_sb[:, ci * 128:(ci + 1) * 128],
                hi[:],
                start=True, stop=True,
            )
        x2b = xpool.tile([128, WCH, QPC], BF16, tag="x2")
        nc.scalar.activation(x2b[:], qdt[:, :, 0:QPC], SQ)     # qd^2
        x3b = xpool.tile([128, WCH, QPC], BF16, tag="x3")
        nc.vector.tensor_mul(x3b[:], qdt[:, :, 0:QPC], x2b[:])  # qd^3
        for k in range(WCH):
            ci = w * WCH + k
            for bi, ((qo, qn), sc) in enumerate(zip(QBLOCKS, scs)):
                nc.tensor.matmul(
                    sc[:],
                    x3b[:, k, qo:qo + qn],
                    s3[:, ci * NS:(ci + 1) * NS],
                    start=False,
                    stop=(ci == NCHUNK - 1 and bi == len(QBLOCKS) - 1),
                )

    # ---- writeback ------------------------------------------------------
    o0 = work.tile([128, NS], F32, tag="o0")
    nc.scalar.activation(o0[:], sc0[:], CP)
    o1 = work.tile([60, NS], F32, tag="o1")
    nc.scalar.activation(o1[:], sc1[:], CP)
    nc.sync.dma_start(out[0:128, :], o0[:])
    nc.sync.dma_start(out[128:QPC, :], o1[:])


_NC_CACHE = {}


def _build_nc():
    if "nc" in _NC_CACHE:
        return _NC_CACHE["nc"]
    nc = bacc.Bacc()
    q = nc.dram_tensor("q", [QPC, CS], F32, kind="ExternalInput")
    smat = nc.dram_tensor("smat", [C, NPAD], BF16, kind="ExternalInput")
    protoT = nc.dram_tensor("protoT", [C, NS], BF16, kind="ExternalInput")
    pm_hi = nc.dram_tensor("pm_hi", [C, NS], BF16, kind="ExternalInput")
    pm_lo = nc.dram_tensor("pm_lo", [C, NS], BF16, kind="ExternalInput")
    ident = nc.dram_tensor("ident", [128, 128], F32, kind="ExternalInput")
    out = nc.dram_tensor("out", [QPC, NS], F32, kind="ExternalOutput")
    with tile.TileContext(nc) as tc:
        _body(tc, q[:], smat[:], protoT[:], pm_hi[:], pm_lo[:], ident[:], out[:])
    nc.compile()
    _NC_CACHE["nc"] = nc
    return nc


LAST_RESULT = None


def kernel(query_feat, support_feat, way_num=5, shot_num=5, query_num=75, **_):
    global LAST_RESULT
    q = np.asarray(query_feat, np.float32)
    s = np.asarray(support_feat, np.float32)
    t, wq, c, h, w = q.shape
    hw = h * w
    nq = t * wq
    way = int(way_num)
    shot = int(shot_num)
    assert c == C and hw == HW and t * way == NS and nq <= NCORES * QPC

    # host prep: prototypes (fp64) + constant matrices
    proto = s.astype(np.float64).reshape(t, way, shot, c, hw).mean(axis=(2, 4))
    proto = proto.reshape(t * way, c)                     # [NS, C]
    iu, ju = np.triu_indices(c, k=1)

    def to_bf(a):
        return np.asarray(a, np.float32).astype(ml_dtypes.bfloat16)

    M1 = c * np.eye(c) - np.ones((c, c))
    protoM = (G * A1 / 2.0) * (M1 @ proto.T)              # [C, NS] fp64
    pm_hi = to_bf(protoM)
    pm_lo = to_bf(protoM - pm_hi.astype(np.float64))
    protoT_b = to_bf(proto.T)

    smat = np.zeros((c, NPAD), np.float32)
    smat[ju, np.arange(NPAIR)] = 1.0
    smat[iu, np.arange(NPAIR)] = -1.0
    smat_b = smat.astype(ml_dtypes.bfloat16)

    ident = np.eye(128, dtype=np.float32)

    qflat = q.reshape(nq, c * hw)
    qpad = np.zeros((NCORES * QPC, c * hw), np.float32)
    qpad[:nq] = qflat

    in_maps = [
        {
            "q": np.ascontiguousarray(qpad[m * QPC:(m + 1) * QPC]),
            "smat": smat_b,
            "protoT": protoT_b,
            "pm_hi": pm_hi,
            "pm_lo": pm_lo,
            "ident": ident,
        }
        for m in range(NCORES)
    ]

    nc = _build_nc()
    # NTFF profiling hooks (antenv.axon_hooks) are absent in this container;
    # force the non-trace path even if BASS_TRACE leaks in from the env.
    os.environ["BASS_NEVER_TRACE"] = "1"
    res = run_bass_kernel_spmd(nc, in_maps, list(range(NCORES)))
    LAST_RESULT = res
    out = np.concatenate([res.results[m]["out"] for m in range(NCORES)], axis=0)
    return np.ascontiguousarray(out[:nq].astype(np.float32))
